# revision 68
# baseline (speedup 1.0000x reference)
"""ChildSum TreeLSTM (N=8192, 4-ary heap tree, H=256, D=300) on 8 trn2 cores.

Design (v3)
-----------
The static tree is processed level-by-level bottom-up. Each core owns 32
level-4 subtrees; the device computes levels 7 and 6 (6827 nodes) and ships
the 4096 level-6 (h, c) states back; the host finishes levels 5..0 (1365
nodes, ~17% of the nodes but a tiny fraction of the FLOPs).

Per-core column layout (XCOLS = 896):
  [ L7 child-major 0:384 | L6-leaf 384:800 | L6-int 800:896 ]

Device tricks:
- x-side gate projections (phase 1) go straight to PSUM; leaf activations
  read PSUM directly (no drain copies). For the 96 internal columns the
  i/o/u projections stay RESIDENT in PSUM and the phase-2 Wh@hs matmuls
  accumulate onto them in place.
- resident PSUM banks are value-zeroed with a DVE memset up front so every
  matmul into them can use start=False (robust to scheduler reordering:
  accumulate-where-written / overwrite-where-not both give the right value
  on a zeroed bank).
- the per-child forget-gate bias gf is added with an identity matmul that
  broadcasts gf into the f PSUM tile (PE work instead of 1x-mode DVE adds).
- L7 states are stored child-major so the 4-child h/c reductions are
  unit-stride bf16 2x-mode adds.
- all inputs ride one priority-ordered sync-HWDGE DMA queue (first-matmul
  dependencies first); activation tables preload at t=0 on the otherwise
  idle scalar queue; L6-leaf states DMA out as soon as they are ready.
"""

import numpy as np
import ml_dtypes

BF16 = ml_dtypes.bfloat16
F32 = np.float32

N = 8192
H = 256
D = 300
K = 4
OUT = 4
NCORES = 8

IPMAX = 86           # internal L6 columns per core (683 total, balanced)
NLEAF = 428          # leaf L6 columns per core (3413 total, padded/balanced)
L7P = K * IPMAX      # 344 L7 columns (child-major: plane c = child c of int j)
NL6 = IPMAX + NLEAF  # 514 L6 columns per core
XCOLS = L7P + NL6 + 2  # 860 (2 dead cols keep every range offset 4-aligned)
KDIM = 304           # xt rows: 300 emb + 1 ones + 3 pad

GATE_MAP = [0, 3, 2, 1]  # packed gate order [i, u, o, f] -> reference indices

NHOST = 1365         # host computes nodes [0, 1365); device supplies L6


def _build_plan():
    """Distribute the 4096 level-6 subtrees (1 L6 node + <=4 L7 children)
    evenly over 8 cores; build per-core column maps. The device computes
    only levels 7 and 6, so L6-subtree granularity is the natural unit."""
    internals = list(range(1365, 2048))          # 683 internal L6 nodes
    leaves = list(range(2048, 5461))             # 3413 leaf L6 nodes
    int_counts = [86, 86, 86, 85, 85, 85, 85, 85]
    leaf_counts = [427, 427, 427, 427, 427, 426, 426, 426]  # padded to NLEAF
    assert sum(int_counts) == len(internals)
    assert sum(leaf_counts) == len(leaves)

    plan = []
    ipos = lpos = 0
    for c in range(NCORES):
        l6i = internals[ipos:ipos + int_counts[c]]
        ipos += int_counts[c]
        l6l = leaves[lpos:lpos + leaf_counts[c]]
        lpos += leaf_counts[c]
        l6i_p = l6i + [-1] * (IPMAX - len(l6i))
        l6l_p = l6l + [-1] * (NLEAF - len(l6l))
        # L7 child-major: col (ch*IPMAX + j) = child ch of internal j
        l7 = np.full((K, IPMAX), -1, dtype=np.int64)
        for j, x in enumerate(l6i):
            for ch in range(K):
                cc = 4 * x + 1 + ch
                l7[ch, j] = cc if cc < N else -1
        # xt column order: [L7 | L6-leaf | L6-int] — internal block last
        cols = np.concatenate([l7.reshape(-1),
                               np.array(l6l_p + l6i_p + [-1, -1],
                                        dtype=np.int64)])
        assert cols.shape == (XCOLS,)
        # state6 column order is [int | leaf]
        l6map = np.array(l6i_p + l6l_p, dtype=np.int64)
        plan.append((cols, len(l6i), l6map))
    return plan


_PLAN = _build_plan()


def _static_tree():
    idx = np.arange(N)[:, None] * K + 1 + np.arange(K)[None, :]
    mask = (idx < N).astype(F32)
    idx = np.where(idx < N, idx, 0).astype(np.int32)
    return idx, mask


_STATIC_IDX, _STATIC_MASK = _static_tree()


FP8 = ml_dtypes.float8_e4m3
SCALE = 64.0         # weight-side scale: PSUM holds 64*(pre-activation)


def _pack_weights(Wx, bx, Wh, bh):
    wx = np.zeros((KDIM, 4 * H), dtype=F32)
    for g, rg in enumerate(GATE_MAP):
        wx[:D, H * g:H * (g + 1)] = np.asarray(Wx[rg], dtype=F32).T
        wx[D, H * g:H * (g + 1)] = (np.asarray(bx[rg], dtype=F32)
                                    + np.asarray(bh[rg], dtype=F32))
    wx *= SCALE
    # rows 0:256 as fp8 [128, 2, 1024] (k-halves stacked on dim 1 for
    # DoubleRow); rows 256:304 stay bf16
    wx8 = np.stack([wx[0:128], wx[128:256]], axis=1).astype(FP8)
    wx2 = wx[256:KDIM].astype(BF16)
    wh = np.zeros((H, 3 * H), dtype=F32)
    for g, rg in enumerate([0, 2, 3]):  # i, o, u
        wh[:, H * g:H * (g + 1)] = np.asarray(Wh[rg], dtype=F32).T
    whf = np.ascontiguousarray(np.asarray(Wh[1], dtype=F32).T)
    ident = np.eye(128, dtype=F32)
    return (wx8, wx2, (wh * SCALE).astype(BF16), (whf * SCALE).astype(BF16),
            ident.astype(BF16))


def _pack_xt(xs, emb_table):
    X = np.asarray(emb_table, dtype=F32)[np.asarray(xs)]
    xt8s, xt2s = [], []
    for cols, _, _ in _PLAN:
        xt = np.zeros((KDIM, XCOLS), dtype=F32)
        real = cols >= 0
        xt[:D, real] = X[cols[real]].T
        xt[D, real] = 1.0
        xt8s.append(np.stack([xt[0:128], xt[128:256]], axis=1).astype(FP8))
        xt2s.append(xt[256:KDIM].astype(BF16))
    return xt8s, xt2s


def _sigmoid(x):
    return (1.0 / (1.0 + np.exp(-x))).astype(F32)


def _log_softmax(x):
    m = np.max(x)
    e = np.exp(x - m)
    return (x - m - np.log(e.sum())).astype(F32)


def _host_top(Hbuf, Cbuf, xs, emb_table, Wx, bx, Wh, bh):
    """Compute tree levels 5..0 (nodes 0..1364) on the host in fp32 numpy."""
    Wx = np.asarray(Wx, dtype=F32)
    bx = np.asarray(bx, dtype=F32)
    Wh = np.asarray(Wh, dtype=F32)
    bh = np.asarray(bh, dtype=F32)
    emb = np.asarray(emb_table, dtype=F32)
    xs = np.asarray(xs)
    for lo, hi in [(341, 1365), (85, 341), (21, 85), (5, 21), (1, 5), (0, 1)]:
        ids = np.arange(lo, hi)
        Xl = emb[xs[ids]]
        gx = np.einsum('ghd,nd->ngh', Wx, Xl).astype(F32) + bx
        cidx = ids[:, None] * K + 1 + np.arange(K)[None, :]
        Hc = Hbuf[cidx]
        Cc = Cbuf[cidx]
        hsum = Hc.sum(1)
        ig = _sigmoid(gx[:, 0] + hsum @ Wh[0].T + bh[0])
        og = _sigmoid(gx[:, 2] + hsum @ Wh[2].T + bh[2])
        ug = np.tanh(gx[:, 3] + hsum @ Wh[3].T + bh[3]).astype(F32)
        f = _sigmoid(gx[:, 1][:, None, :] + Hc @ Wh[1].T + bh[1])
        cc = ig * ug + (f * Cc).sum(1)
        hh = og * np.tanh(cc).astype(F32)
        Hbuf[ids] = hh
        Cbuf[ids] = cc
    return Hbuf[0]


def simulate_cores_numpy(inputs):
    """Numpy emulation of the device data layout & schedule (fp32 math).

    Returns (Hbuf, Cbuf) [5461, H] filled for nodes [1365, 5461) — validates
    the plan/layout without hardware.
    """
    xs = np.asarray(inputs["xs"])
    wx8, wx2, wh, whf, _ = _pack_weights(inputs["Wx"], inputs["bx"],
                                         inputs["Wh"], inputs["bh"])
    wx = np.concatenate([wx8[:, 0].astype(F32), wx8[:, 1].astype(F32),
                         wx2.astype(F32)]) / SCALE
    wh = wh.astype(F32) / SCALE
    whf = whf.astype(F32) / SCALE
    xt8s, xt2s = _pack_xt(xs, inputs["emb_table"])
    Hbuf = np.zeros((5461, H), dtype=F32)
    Cbuf = np.zeros((5461, H), dtype=F32)
    for c in range(NCORES):
        cols, wc, l6map = _PLAN[c]
        xt = np.concatenate([xt8s[c][:, 0].astype(F32),
                             xt8s[c][:, 1].astype(F32),
                             xt2s[c].astype(F32)])
        G = wx[:301].T @ xt[:301]                    # [1024, XCOLS]
        gi, gu, go, gf = (G[0:H], G[H:2*H], G[2*H:3*H], G[3*H:4*H])

        def leaf(sl):
            cc = _sigmoid(gi[:, sl]) * np.tanh(gu[:, sl]).astype(F32)
            hh = _sigmoid(go[:, sl]) * np.tanh(cc).astype(F32)
            return hh, cc

        H7, C7 = leaf(slice(0, L7P))                 # [H, 384] child-major
        H6 = np.zeros((H, NL6), dtype=F32)
        C6 = np.zeros((H, NL6), dtype=F32)
        H6[:, IPMAX:], C6[:, IPMAX:] = leaf(slice(L7P, L7P + NLEAF))

        # internal chunk (child-major children, 96 cols)
        sl = slice(L7P + NLEAF, L7P + NLEAF + IPMAX)
        ip = IPMAX
        hs = H7.reshape(H, K, ip).sum(1)
        gfr = np.tile(gf[:, sl], (1, K))             # [H, K*ip]
        A = wh.T @ hs                                # [768, ip]
        ig = _sigmoid(gi[:, sl] + A[0:H])
        og = _sigmoid(go[:, sl] + A[H:2*H])
        ug = np.tanh(gu[:, sl] + A[2*H:3*H]).astype(F32)
        FA = whf.T @ H7 + gfr
        FS = _sigmoid(FA) * C7
        csum = FS.reshape(H, K, ip).sum(1)
        cc = ig * ug + csum
        hh = og * np.tanh(cc).astype(F32)
        H6[:, :IPMAX], C6[:, :IPMAX] = hh, cc
        valid = l6map >= 0
        Hbuf[l6map[valid]] = H6.T[valid]
        Cbuf[l6map[valid]] = C6.T[valid]
    return Hbuf, Cbuf


# ----------------------------------------------------------------------------
# Bass device program
# ----------------------------------------------------------------------------

_COMPILED = None


def _build_device_program():
    import contextlib

    import concourse.bacc as bacc
    import concourse.tile as tile
    import concourse.mybir as mybir

    f32 = mybir.dt.float32
    bf16 = mybir.dt.bfloat16
    fp8 = mybir.dt.float8e4
    DR = mybir.MatmulPerfMode.DoubleRow
    Sig = mybir.ActivationFunctionType.Sigmoid
    Tanh = mybir.ActivationFunctionType.Tanh
    INV = 1.0 / SCALE

    nc = bacc.Bacc("TRN2", target_bir_lowering=False, debug=False,
                   num_devices=NCORES, enable_partition_id=False,
                   enable_asserts=False)

    xt8_d = nc.dram_tensor("xt8", [128, 2, XCOLS], fp8, kind="ExternalInput")
    xt2_d = nc.dram_tensor("xt2", [KDIM - 256, XCOLS], bf16,
                           kind="ExternalInput")
    wx8_d = nc.dram_tensor("wx8", [128, 2, 4 * H], fp8, kind="ExternalInput")
    wx2_d = nc.dram_tensor("wx2", [KDIM - 256, 4 * H], bf16,
                           kind="ExternalInput")
    wh_d = nc.dram_tensor("wh", [H, 3 * H], bf16, kind="ExternalInput")
    whf_d = nc.dram_tensor("whf", [H, H], bf16, kind="ExternalInput")
    id_d = nc.dram_tensor("ident", [128, 128], bf16, kind="ExternalInput")
    out_h_d = nc.dram_tensor("out_h", [128, 2, NL6], bf16,
                             kind="ExternalOutput")
    out_c_d = nc.dram_tensor("out_c", [128, 2, NL6], bf16,
                             kind="ExternalOutput")

    R0 = (0, L7P)              # L7 leaves
    R1 = (L7P, L7P + NLEAF)    # L6 leaves
    RI = (L7P + NLEAF, L7P + NLEAF + IPMAX)  # internal (L6i)
    ip = IPMAX

    with tile.TileContext(nc) as tc:
        with contextlib.ExitStack() as ctx:
            inp = ctx.enter_context(tc.tile_pool(name="inp", bufs=1))
            st = ctx.enter_context(tc.tile_pool(name="state", bufs=1))
            wk = ctx.enter_context(tc.tile_pool(name="work", bufs=2))
            pres = ctx.enter_context(
                tc.tile_pool(name="pres", bufs=1, space="PSUM"))
            pstr = ctx.enter_context(
                tc.tile_pool(name="pstr", bufs=2, space="PSUM"))
            pwarm = ctx.enter_context(
                tc.tile_pool(name="pwarm", bufs=1, space="PSUM"))

            # ---- input SBUF tiles
            xt8_s = inp.tile([128, 2, XCOLS], fp8, tag="xt8", name="xt8")
            xt2_s = inp.tile([KDIM - 256, XCOLS], bf16, tag="xt2", name="xt2")
            wx8_s = inp.tile([128, 2, 4 * H], fp8, tag="wx8", name="wx8")
            wx2_s = inp.tile([KDIM - 256, 4 * H], bf16, tag="wx2", name="wx2")
            wh_s = []
            whf_s = []
            for k in range(2):
                wh_s.append(inp.tile([128, 3 * H], bf16, tag=f"wh{k}",
                                     name=f"wh{k}"))
                whf_s.append(inp.tile([128, H], bf16, tag=f"whf{k}",
                                      name=f"whf{k}"))
            id_s = inp.tile([128, 128], bf16, tag="ident", name="ident")

            # ---- DMA in: ONE priority-ordered HWDGE queue (sync) so the
            # earliest-needed pieces get full HBM bandwidth, no contention.
            nc.sync.dma_start(out=wx8_s[:, :, 0:512], in_=wx8_d[:, :, 0:512])
            nc.sync.dma_start(out=xt8_s[:], in_=xt8_d[:, :, :])
            nc.sync.dma_start(out=xt2_s[:], in_=xt2_d[:, :])
            nc.sync.dma_start(out=wx2_s[:], in_=wx2_d[:, :])
            nc.sync.dma_start(out=wx8_s[:, :, 512:1024],
                              in_=wx8_d[:, :, 512:1024])

            # ---- activation-table preload on the scalar queue (its table
            # DMA overlaps the input transfers' tail, before any real ACT)
            scr = wk.tile([128, 8], f32, tag="scr", name="scr")
            nc.vector.memset(scr[:], 0.0)
            nc.scalar.activation(scr[:], scr[:], Sig)
            nc.scalar.activation(scr[:], scr[:], Tanh)



            # h-side weights + identity ride the scalar queue behind the
            # table preloads (needed only when the L6i chunk starts)
            for k in range(2):
                nc.scalar.dma_start(out=whf_s[k][:],
                                    in_=whf_d[128*k:128*(k+1), :])
                nc.scalar.dma_start(out=wh_s[k][:],
                                    in_=wh_d[128*k:128*(k+1), :])
            nc.scalar.dma_start(out=id_s[:], in_=id_d[:, :])

            # ---- persistent state tiles (bf16)
            SH7 = st.tile([128, 2, L7P], bf16, tag="sh7", name="sh7")
            SC7 = st.tile([128, 2, L7P], bf16, tag="sc7", name="sc7")
            SH6 = st.tile([128, 2, NL6], bf16, tag="sh6", name="sh6")
            SC6 = st.tile([128, 2, NL6], bf16, tag="sc6", name="sc6")

            # ---- resident PSUM for internal-column gate pre-activations.
            # Value-zeroed up front; every matmul into them uses start=False
            # (accumulate-where-written / overwrite-where-not — both correct
            # on a zeroed bank regardless of scheduler order).
            res_io = pres.tile([128, 2, 2, ip], f32, tag="rio", name="rio")
            res_u = pres.tile([128, 2, ip], f32, tag="ru", name="ru")
            gf_ps = pres.tile([128, 2, ip], f32, tag="rf", name="rf")
            nc.vector.memset(res_io[:], 0.0)
            nc.vector.memset(res_u[:], 0.0)
            nc.vector.memset(gf_ps[:], 0.0)
            gf_s = st.tile([128, 2, ip], bf16, tag="gfs", name="gfs")

            def mm(out, lhsT, rhs, start, stop, pm=None):
                nc.tensor.matmul(out, lhsT, rhs, start=start, stop=stop,
                                 skip_group_check=True, perf_mode=pm)

            # --- phase-1 helpers. Packed gate order [i, u, o, f]; each
            # (gate, phi) is an fp8-DoubleRow matmul over emb rows 0:256
            # plus a 45-row bf16 tail. The two phi tails of a gate are
            # row-packed into disjoint PE row-groups (rows 0:48 / 64:112 of
            # the duplicated xt2/wx2 tiles) so they run concurrently.
            def p1_gate_outs(out0, out1, g, a, b, start):
                col = 256 * g
                mm(out0, wx8_s[:, :, col:col + 128], xt8_s[:, :, a:b],
                   start=start, stop=False, pm=DR)
                mm(out1, wx8_s[:, :, col + 128:col + 256], xt8_s[:, :, a:b],
                   start=start, stop=False, pm=DR)
                mm(out0, wx2_s[0:48, col:col + 128], xt2_s[0:48, a:b],
                   start=False, stop=True)
                mm(out1, wx2_s[0:48, col + 128:col + 256],
                   xt2_s[0:48, a:b], start=False, stop=True)

            # one gate over leaf cols [a,b) into tile P
            def p1_gate(P, g, a, b):
                n = b - a
                p1_gate_outs(P[:, 0, 0:n], P[:, 1, 0:n], g, a, b, start=True)

            # --- phase-1 into resident banks (internal cols, all 4 gates);
            # banks are pre-zeroed so every matmul uses start=False
            def p1_resident():
                a, b = RI
                p1_gate_outs(res_io[:, 0, 0, :], res_io[:, 0, 1, :], 0, a, b,
                             start=False)
                p1_gate_outs(res_io[:, 1, 0, :], res_io[:, 1, 1, :], 2, a, b,
                             start=False)
                p1_gate_outs(res_u[:, 0, :], res_u[:, 1, :], 1, a, b,
                             start=False)
                p1_gate_outs(gf_ps[:, 0, :], gf_ps[:, 1, :], 3, a, b,
                             start=False)
                # gf (x64) to SBUF bf16 for the later broadcast matmul
                nc.vector.tensor_copy(gf_s[:], gf_ps[:])

            # --- leaf ranges: PSUM gate tiles -> activations -> states
            def leaf_mm_gate(g, a, b, name):
                P = pstr.tile([128, 2, 512], f32, tag="lps", name=name)
                p1_gate(P, g, a, b)
                return P

            def leaf_act(P, n, func, tag, off):
                G = wk.tile([128, 2, n], bf16, tag=tag, name=f"{tag}{off}")
                nc.scalar.activation(G[:], P[:, :, 0:n], func, scale=INV)
                return G

            def leaf_range(a, b, SH, SC, off):
                # gate order i, u, o: c and tanh(c) complete while the
                # o-gate matmuls still stream, shortening the h tail
                n = b - a
                Pi = leaf_mm_gate(0, a, b, f"pi{a}")
                Pu = leaf_mm_gate(1, a, b, f"pu{a}")
                GI = leaf_act(Pi, n, Sig, "gi", off)
                GU = leaf_act(Pu, n, Tanh, "gu", off)
                Cd = SC[:, :, off:off + n]
                nc.vector.tensor_mul(Cd, GI[:], GU[:])
                TC = wk.tile([128, 2, n], bf16, tag="tc", name=f"tc{off}")
                nc.scalar.activation(TC[:], Cd, Tanh)
                # o-gate phi-split: h for phi0 lands while phi1 still runs
                Po = leaf_mm_gate(2, a, b, f"po{a}")
                GO = wk.tile([128, 2, n], bf16, tag="go", name=f"go{off}")
                for phi in range(2):
                    nc.scalar.activation(GO[:, phi], Po[:, phi, 0:n], Sig,
                                         scale=INV)
                    nc.vector.tensor_mul(SH[:, phi, off:off + n],
                                         GO[:, phi], TC[:, phi])

            # --- the one internal chunk: L7 (child-major) -> L6[0:96]
            # part 1: f-gate matmuls + child h-sum (needs only SH7)
            def chunk_part1():
                nf = K * ip  # 384
                Pf = pstr.tile([128, 2, 512], f32, tag="lps", name="pf")
                for phi in range(2):
                    for k in range(2):
                        mm(Pf[:, phi, 0:nf],
                           whf_s[k][:, 128 * phi:128 * phi + 128],
                           SH7[:, k, :], start=(k == 0), stop=False)
                    gbr = gf_s[:, phi, :][:, None, :].broadcast_to(
                        [128, K, ip])
                    mm(Pf[:, phi, 0:nf], id_s[:, 0:128], gbr,
                       start=False, stop=True)
                # hs = sum of 4 children (child-major: unit-stride adds,
                # split across the vector + gpsimd engines)
                cv = SH7.rearrange("p t (c j) -> p t c j", c=K)
                A = wk.tile([128, 2, ip], bf16, tag="ha", name="ha")
                nc.vector.tensor_add(A[:], cv[:, :, 0, :], cv[:, :, 1, :])
                B = wk.tile([128, 2, ip], bf16, tag="hb", name="hb")
                nc.gpsimd.tensor_add(B[:], cv[:, :, 2, :], cv[:, :, 3, :])
                hs = wk.tile([128, 2, ip], bf16, tag="hs", name="hs")
                nc.vector.tensor_add(hs[:], A[:], B[:])
                return Pf, hs

            # part 2: i/o/u h-side matmuls + activations + cell update
            def chunk_part2(Pf, hs):
                nf = K * ip
                for gi_ in range(2):
                    for phi in range(2):
                        for k in range(2):
                            mm(res_io[:, gi_, phi, :],
                               wh_s[k][:, 256 * gi_ + 128 * phi:
                                       256 * gi_ + 128 * phi + 128],
                               hs[:, k, :], start=False, stop=(k == 1))
                for phi in range(2):
                    for k in range(2):
                        mm(res_u[:, phi, :],
                           wh_s[k][:, 512 + 128 * phi:512 + 128 * phi + 128],
                           hs[:, k, :], start=False, stop=(k == 1))
                # activations
                SF = wk.tile([128, 2, nf], bf16, tag="sf", name="sf")
                nc.scalar.activation(SF[:], Pf[:, :, 0:nf], Sig, scale=INV)
                Gio = wk.tile([128, 2, 2, ip], bf16, tag="gio", name="gio")
                nc.scalar.activation(Gio[:], res_io[:], Sig, scale=INV)
                GU = wk.tile([128, 2, ip], bf16, tag="gu", name="cgu")
                nc.scalar.activation(GU[:], res_u[:], Tanh, scale=INV)
                # FS = sigmoid(FA) * c_child ; csum = sum over 4 children
                FS = wk.tile([128, 2, nf], bf16, tag="fs", name="fs")
                nc.vector.tensor_mul(FS[:], SF[:], SC7[:])
                fv = FS.rearrange("p t (c j) -> p t c j", c=K)
                CA = wk.tile([128, 2, ip], bf16, tag="ca", name="ca")
                nc.vector.tensor_add(CA[:], fv[:, :, 0, :], fv[:, :, 1, :])
                CB = wk.tile([128, 2, ip], bf16, tag="cb", name="cb")
                nc.gpsimd.tensor_add(CB[:], fv[:, :, 2, :], fv[:, :, 3, :])
                # c = ig*ug + (CA + CB) ; h = og*tanh(c)
                t1 = wk.tile([128, 2, ip], bf16, tag="t1", name="t1")
                nc.vector.tensor_mul(t1[:], Gio[:, 0], GU[:])
                t2 = wk.tile([128, 2, ip], bf16, tag="t2", name="t2")
                nc.vector.tensor_add(t2[:], t1[:], CA[:])
                Cd = SC6[:, :, 0:ip]
                nc.vector.tensor_add(Cd, t2[:], CB[:])
                nc.scalar.dma_start(out=out_c_d[:, :, 0:IPMAX],
                                    in_=SC6[:, :, 0:IPMAX])
                TC = wk.tile([128, 2, ip], bf16, tag="tc2", name="tc2")
                nc.scalar.activation(TC[:], Cd, Tanh)
                nc.vector.tensor_mul(SH6[:, :, 0:ip], Gio[:, 1], TC[:])
                nc.scalar.dma_start(out=out_h_d[:, :, 0:IPMAX],
                                    in_=SH6[:, :, 0:IPMAX])

            # ================= program order =================
            # R0 -> RI -> R1-i -> chunk f/hs -> R1-o (fills the hs wait)
            # -> chunk i/o/u + cell -> R1-u + states -> DMA out (sync queue)
            leaf_range(R0[0], R0[1], SH7, SC7, 0)
            p1_resident()
            n1 = R1[1] - R1[0]
            Pi1 = leaf_mm_gate(0, R1[0], R1[1], "pi1")
            GI1 = leaf_act(Pi1, n1, Sig, "gi", IPMAX)
            Pu1 = leaf_mm_gate(1, R1[0], R1[1], "pu1")
            Pf, hs = chunk_part1()
            GU1 = leaf_act(Pu1, n1, Tanh, "gu", IPMAX)
            Cd1 = SC6[:, :, IPMAX:NL6]
            nc.vector.tensor_mul(Cd1, GI1[:], GU1[:])
            TC1 = wk.tile([128, 2, n1], bf16, tag="tc", name="tc96")
            nc.sync.dma_start(out=out_c_d[:, :, IPMAX:NL6],
                              in_=SC6[:, :, IPMAX:NL6])
            nc.scalar.activation(TC1[:], Cd1, Tanh)
            chunk_part2(Pf, hs)
            Po1 = leaf_mm_gate(2, R1[0], R1[1], "po1")
            GO1 = wk.tile([128, 2, n1], bf16, tag="go", name="go96")
            for phi in range(2):
                nc.scalar.activation(GO1[:, phi], Po1[:, phi, 0:n1], Sig,
                                     scale=INV)
                nc.vector.tensor_mul(SH6[:, phi, IPMAX:NL6],
                                     GO1[:, phi], TC1[:, phi])
                nc.sync.dma_start(out=out_h_d[:, phi, IPMAX:NL6],
                                  in_=SH6[:, phi, IPMAX:NL6])

    nc.compile()
    return nc


def _get_compiled():
    global _COMPILED
    if _COMPILED is None:
        _COMPILED = _build_device_program()
    return _COMPILED


def _numpy_fallback(xs, child_idx, child_mask, emb_table, Wx, bx, Wh, bh,
                    Wout, bout):
    """Exact sequential scan; only used if the tree isn't the static heap."""
    X = np.asarray(emb_table, dtype=F32)[np.asarray(xs)]
    Wx = np.asarray(Wx, dtype=F32)
    Wh = np.asarray(Wh, dtype=F32)
    bx = np.asarray(bx, dtype=F32)
    bh = np.asarray(bh, dtype=F32)
    gx = np.einsum('ghd,nd->ngh', Wx, X).astype(F32) + bx
    Hb = np.zeros((N, H), dtype=F32)
    Cb = np.zeros((N, H), dtype=F32)
    ci = np.asarray(child_idx)
    cm = np.asarray(child_mask, dtype=F32)
    for i in range(N - 1, -1, -1):
        idx = ci[i]
        m = cm[i][:, None]
        Hc = Hb[idx] * m
        Cc = Cb[idx] * m
        hsum = Hc.sum(0)
        g = gx[i]
        ig = _sigmoid(g[0] + Wh[0] @ hsum + bh[0])
        og = _sigmoid(g[2] + Wh[2] @ hsum + bh[2])
        ug = np.tanh(g[3] + Wh[3] @ hsum + bh[3]).astype(F32)
        f = _sigmoid(g[1] + Hc @ Wh[1].T + bh[1])
        c = ig * ug + (f * Cc).sum(0)
        Hb[i] = og * np.tanh(c).astype(F32)
        Cb[i] = c
    logits = np.asarray(Wout, dtype=F32) @ Hb[0] + np.asarray(bout, dtype=F32)
    return _log_softmax(logits)


def kernel(xs, child_idx, child_mask, emb_table, Wx, bx, Wh, bh, Wout, bout):
    xs = np.asarray(xs)
    if not (np.array_equal(np.asarray(child_idx), _STATIC_IDX)
            and np.array_equal(np.asarray(child_mask, dtype=F32),
                               _STATIC_MASK)):
        return _numpy_fallback(xs, child_idx, child_mask, emb_table, Wx, bx,
                               Wh, bh, Wout, bout)

    from concourse.bass_utils import run_bass_kernel_spmd

    wx8, wx2, wh, whf, ident = _pack_weights(Wx, bx, Wh, bh)
    xt8s, xt2s = _pack_xt(xs, emb_table)
    in_maps = [{"xt8": xt8s[c], "xt2": xt2s[c], "wx8": wx8, "wx2": wx2,
                "wh": wh, "whf": whf, "ident": ident}
               for c in range(NCORES)]
    nc = _get_compiled()
    res = run_bass_kernel_spmd(nc, in_maps, core_ids=list(range(NCORES)))

    Hbuf = np.zeros((5461, H), dtype=F32)
    Cbuf = np.zeros((5461, H), dtype=F32)
    for c in range(NCORES):
        _, _, l6map = _PLAN[c]
        oh = np.asarray(res.results[c]["out_h"], dtype=F32)  # [128, 2, NL6]
        oc = np.asarray(res.results[c]["out_c"], dtype=F32)
        valid = l6map >= 0
        Hbuf[l6map[valid]] = np.concatenate(
            [oh[:, 0, :], oh[:, 1, :]], axis=0).T[valid]
        Cbuf[l6map[valid]] = np.concatenate(
            [oc[:, 0, :], oc[:, 1, :]], axis=0).T[valid]

    h0 = _host_top(Hbuf, Cbuf, xs, emb_table, Wx, bx, Wh, bh)
    logits = np.asarray(Wout, dtype=F32) @ h0 + np.asarray(bout, dtype=F32)
    return _log_softmax(logits)


# revision 69
# speedup vs baseline: 1.0347x; 1.0347x over previous
"""ChildSum TreeLSTM (N=8192, 4-ary heap tree, H=256, D=300) on 8 trn2 cores.

Design (v3)
-----------
The static tree is processed level-by-level bottom-up. Each core owns 32
level-4 subtrees; the device computes levels 7 and 6 (6827 nodes) and ships
the 4096 level-6 (h, c) states back; the host finishes levels 5..0 (1365
nodes, ~17% of the nodes but a tiny fraction of the FLOPs).

Per-core column layout (XCOLS = 896):
  [ L7 child-major 0:384 | L6-leaf 384:800 | L6-int 800:896 ]

Device tricks:
- x-side gate projections (phase 1) go straight to PSUM; leaf activations
  read PSUM directly (no drain copies). For the 96 internal columns the
  i/o/u projections stay RESIDENT in PSUM and the phase-2 Wh@hs matmuls
  accumulate onto them in place.
- resident PSUM banks are value-zeroed with a DVE memset up front so every
  matmul into them can use start=False (robust to scheduler reordering:
  accumulate-where-written / overwrite-where-not both give the right value
  on a zeroed bank).
- the per-child forget-gate bias gf is added with an identity matmul that
  broadcasts gf into the f PSUM tile (PE work instead of 1x-mode DVE adds).
- L7 states are stored child-major so the 4-child h/c reductions are
  unit-stride bf16 2x-mode adds.
- all inputs ride one priority-ordered sync-HWDGE DMA queue (first-matmul
  dependencies first); activation tables preload at t=0 on the otherwise
  idle scalar queue; L6-leaf states DMA out as soon as they are ready.
"""

import numpy as np
import ml_dtypes

BF16 = ml_dtypes.bfloat16
F32 = np.float32

N = 8192
H = 256
D = 300
K = 4
OUT = 4
NCORES = 8

IPMAX = 86           # internal L6 columns per core (683 total, balanced)
NLEAF = 428          # leaf L6 columns per core (3413 total, padded/balanced)
L7P = K * IPMAX      # 344 L7 columns (child-major: plane c = child c of int j)
NL6 = IPMAX + NLEAF  # 514 L6 columns per core
XCOLS = L7P + NL6 + 2  # 860 (2 dead cols keep every range offset 4-aligned)
KDIM = 304           # xt rows: 300 emb + 1 ones + 3 pad

GATE_MAP = [0, 3, 2, 1]  # packed gate order [i, u, o, f] -> reference indices

NHOST = 1365         # host computes nodes [0, 1365); device supplies L6


def _build_plan():
    """Distribute the 4096 level-6 subtrees (1 L6 node + <=4 L7 children)
    evenly over 8 cores; build per-core column maps. The device computes
    only levels 7 and 6, so L6-subtree granularity is the natural unit."""
    internals = list(range(1365, 2048))          # 683 internal L6 nodes
    leaves = list(range(2048, 5461))             # 3413 leaf L6 nodes
    int_counts = [86, 86, 86, 85, 85, 85, 85, 85]
    leaf_counts = [427, 427, 427, 427, 427, 426, 426, 426]  # padded to NLEAF
    assert sum(int_counts) == len(internals)
    assert sum(leaf_counts) == len(leaves)

    plan = []
    ipos = lpos = 0
    for c in range(NCORES):
        l6i = internals[ipos:ipos + int_counts[c]]
        ipos += int_counts[c]
        l6l = leaves[lpos:lpos + leaf_counts[c]]
        lpos += leaf_counts[c]
        l6i_p = l6i + [-1] * (IPMAX - len(l6i))
        l6l_p = l6l + [-1] * (NLEAF - len(l6l))
        # L7 child-major: col (ch*IPMAX + j) = child ch of internal j
        l7 = np.full((K, IPMAX), -1, dtype=np.int64)
        for j, x in enumerate(l6i):
            for ch in range(K):
                cc = 4 * x + 1 + ch
                l7[ch, j] = cc if cc < N else -1
        # xt column order: [L7 | L6-leaf | L6-int] — internal block last
        cols = np.concatenate([l7.reshape(-1),
                               np.array(l6l_p + l6i_p + [-1, -1],
                                        dtype=np.int64)])
        assert cols.shape == (XCOLS,)
        # state6 column order is [int | leaf]
        l6map = np.array(l6i_p + l6l_p, dtype=np.int64)
        plan.append((cols, len(l6i), l6map))
    return plan


_PLAN = _build_plan()


def _static_tree():
    idx = np.arange(N)[:, None] * K + 1 + np.arange(K)[None, :]
    mask = (idx < N).astype(F32)
    idx = np.where(idx < N, idx, 0).astype(np.int32)
    return idx, mask


_STATIC_IDX, _STATIC_MASK = _static_tree()


FP8 = ml_dtypes.float8_e4m3
SCALE = 64.0         # weight-side scale: PSUM holds 64*(pre-activation)


def _pack_weights(Wx, bx, Wh, bh):
    wx = np.zeros((KDIM, 4 * H), dtype=F32)
    for g, rg in enumerate(GATE_MAP):
        wx[:D, H * g:H * (g + 1)] = np.asarray(Wx[rg], dtype=F32).T
        wx[D, H * g:H * (g + 1)] = (np.asarray(bx[rg], dtype=F32)
                                    + np.asarray(bh[rg], dtype=F32))
    wx *= SCALE
    # rows 0:256 as fp8 [128, 2, 1024] (k-halves stacked on dim 1 for
    # DoubleRow); rows 256:304 stay bf16
    wx8 = np.stack([wx[0:128], wx[128:256]], axis=1).astype(FP8)
    wx2 = wx[256:KDIM].astype(BF16)
    wh = np.zeros((H, 3 * H), dtype=F32)
    for g, rg in enumerate([0, 2, 3]):  # i, o, u
        wh[:, H * g:H * (g + 1)] = np.asarray(Wh[rg], dtype=F32).T
    whf = np.ascontiguousarray(np.asarray(Wh[1], dtype=F32).T)
    ident = np.eye(128, dtype=F32)
    return (wx8, wx2, (wh * SCALE).astype(BF16), (whf * SCALE).astype(BF16),
            ident.astype(BF16))


def _pack_xt(xs, emb_table):
    X = np.asarray(emb_table, dtype=F32)[np.asarray(xs)]
    xt8s, xt2s = [], []
    for cols, _, _ in _PLAN:
        xt = np.zeros((KDIM, XCOLS), dtype=F32)
        real = cols >= 0
        xt[:D, real] = X[cols[real]].T
        xt[D, real] = 1.0
        xt8s.append(np.stack([xt[0:128], xt[128:256]], axis=1).astype(FP8))
        xt2s.append(xt[256:KDIM].astype(BF16))
    return xt8s, xt2s


def _sigmoid(x):
    return (1.0 / (1.0 + np.exp(-x))).astype(F32)


def _log_softmax(x):
    m = np.max(x)
    e = np.exp(x - m)
    return (x - m - np.log(e.sum())).astype(F32)


def _host_top(Hbuf, Cbuf, xs, emb_table, Wx, bx, Wh, bh):
    """Compute tree levels 5..0 (nodes 0..1364) on the host in fp32 numpy."""
    Wx = np.asarray(Wx, dtype=F32)
    bx = np.asarray(bx, dtype=F32)
    Wh = np.asarray(Wh, dtype=F32)
    bh = np.asarray(bh, dtype=F32)
    emb = np.asarray(emb_table, dtype=F32)
    xs = np.asarray(xs)
    for lo, hi in [(341, 1365), (85, 341), (21, 85), (5, 21), (1, 5), (0, 1)]:
        ids = np.arange(lo, hi)
        Xl = emb[xs[ids]]
        gx = np.einsum('ghd,nd->ngh', Wx, Xl).astype(F32) + bx
        cidx = ids[:, None] * K + 1 + np.arange(K)[None, :]
        Hc = Hbuf[cidx]
        Cc = Cbuf[cidx]
        hsum = Hc.sum(1)
        ig = _sigmoid(gx[:, 0] + hsum @ Wh[0].T + bh[0])
        og = _sigmoid(gx[:, 2] + hsum @ Wh[2].T + bh[2])
        ug = np.tanh(gx[:, 3] + hsum @ Wh[3].T + bh[3]).astype(F32)
        f = _sigmoid(gx[:, 1][:, None, :] + Hc @ Wh[1].T + bh[1])
        cc = ig * ug + (f * Cc).sum(1)
        hh = og * np.tanh(cc).astype(F32)
        Hbuf[ids] = hh
        Cbuf[ids] = cc
    return Hbuf[0]


def simulate_cores_numpy(inputs):
    """Numpy emulation of the device data layout & schedule (fp32 math).

    Returns (Hbuf, Cbuf) [5461, H] filled for nodes [1365, 5461) — validates
    the plan/layout without hardware.
    """
    xs = np.asarray(inputs["xs"])
    wx8, wx2, wh, whf, _ = _pack_weights(inputs["Wx"], inputs["bx"],
                                         inputs["Wh"], inputs["bh"])
    wx = np.concatenate([wx8[:, 0].astype(F32), wx8[:, 1].astype(F32),
                         wx2.astype(F32)]) / SCALE
    wh = wh.astype(F32) / SCALE
    whf = whf.astype(F32) / SCALE
    xt8s, xt2s = _pack_xt(xs, inputs["emb_table"])
    Hbuf = np.zeros((5461, H), dtype=F32)
    Cbuf = np.zeros((5461, H), dtype=F32)
    for c in range(NCORES):
        cols, wc, l6map = _PLAN[c]
        xt = np.concatenate([xt8s[c][:, 0].astype(F32),
                             xt8s[c][:, 1].astype(F32),
                             xt2s[c].astype(F32)])
        G = wx[:301].T @ xt[:301]                    # [1024, XCOLS]
        gi, gu, go, gf = (G[0:H], G[H:2*H], G[2*H:3*H], G[3*H:4*H])

        def leaf(sl):
            cc = _sigmoid(gi[:, sl]) * np.tanh(gu[:, sl]).astype(F32)
            hh = _sigmoid(go[:, sl]) * np.tanh(cc).astype(F32)
            return hh, cc

        H7, C7 = leaf(slice(0, L7P))                 # [H, 384] child-major
        H6 = np.zeros((H, NL6), dtype=F32)
        C6 = np.zeros((H, NL6), dtype=F32)
        H6[:, IPMAX:], C6[:, IPMAX:] = leaf(slice(L7P, L7P + NLEAF))

        # internal chunk (child-major children, 96 cols)
        sl = slice(L7P + NLEAF, L7P + NLEAF + IPMAX)
        ip = IPMAX
        hs = H7.reshape(H, K, ip).sum(1)
        gfr = np.tile(gf[:, sl], (1, K))             # [H, K*ip]
        A = wh.T @ hs                                # [768, ip]
        ig = _sigmoid(gi[:, sl] + A[0:H])
        og = _sigmoid(go[:, sl] + A[H:2*H])
        ug = np.tanh(gu[:, sl] + A[2*H:3*H]).astype(F32)
        FA = whf.T @ H7 + gfr
        FS = _sigmoid(FA) * C7
        csum = FS.reshape(H, K, ip).sum(1)
        cc = ig * ug + csum
        hh = og * np.tanh(cc).astype(F32)
        H6[:, :IPMAX], C6[:, :IPMAX] = hh, cc
        valid = l6map >= 0
        Hbuf[l6map[valid]] = H6.T[valid]
        Cbuf[l6map[valid]] = C6.T[valid]
    return Hbuf, Cbuf


# ----------------------------------------------------------------------------
# Bass device program
# ----------------------------------------------------------------------------

_COMPILED = None


def _build_device_program():
    import contextlib

    import concourse.bacc as bacc
    import concourse.tile as tile
    import concourse.mybir as mybir

    f32 = mybir.dt.float32
    bf16 = mybir.dt.bfloat16
    fp8 = mybir.dt.float8e4
    DR = mybir.MatmulPerfMode.DoubleRow
    Sig = mybir.ActivationFunctionType.Sigmoid
    Tanh = mybir.ActivationFunctionType.Tanh
    INV = 1.0 / SCALE

    nc = bacc.Bacc("TRN2", target_bir_lowering=False, debug=False,
                   num_devices=NCORES, enable_partition_id=False,
                   enable_asserts=False)

    xt8_d = nc.dram_tensor("xt8", [128, 2, XCOLS], fp8, kind="ExternalInput")
    xt2_d = nc.dram_tensor("xt2", [KDIM - 256, XCOLS], bf16,
                           kind="ExternalInput")
    wx8_d = nc.dram_tensor("wx8", [128, 2, 4 * H], fp8, kind="ExternalInput")
    wx2_d = nc.dram_tensor("wx2", [KDIM - 256, 4 * H], bf16,
                           kind="ExternalInput")
    wh_d = nc.dram_tensor("wh", [H, 3 * H], bf16, kind="ExternalInput")
    whf_d = nc.dram_tensor("whf", [H, H], bf16, kind="ExternalInput")
    id_d = nc.dram_tensor("ident", [128, 128], bf16, kind="ExternalInput")
    out_h_d = nc.dram_tensor("out_h", [128, 2, NL6], bf16,
                             kind="ExternalOutput")
    out_c_d = nc.dram_tensor("out_c", [128, 2, NL6], bf16,
                             kind="ExternalOutput")

    R0 = (0, L7P)              # L7 leaves
    R1 = (L7P, L7P + NLEAF)    # L6 leaves
    RI = (L7P + NLEAF, L7P + NLEAF + IPMAX)  # internal (L6i)
    ip = IPMAX

    with tile.TileContext(nc) as tc:
        with contextlib.ExitStack() as ctx:
            inp = ctx.enter_context(tc.tile_pool(name="inp", bufs=1))
            st = ctx.enter_context(tc.tile_pool(name="state", bufs=1))
            wk = ctx.enter_context(tc.tile_pool(name="work", bufs=2))
            pres = ctx.enter_context(
                tc.tile_pool(name="pres", bufs=1, space="PSUM"))
            pstr = ctx.enter_context(
                tc.tile_pool(name="pstr", bufs=2, space="PSUM"))
            pwarm = ctx.enter_context(
                tc.tile_pool(name="pwarm", bufs=1, space="PSUM"))

            # ---- input SBUF tiles
            xt8_s = inp.tile([128, 2, XCOLS], fp8, tag="xt8", name="xt8")
            xt2_s = inp.tile([KDIM - 256, XCOLS], bf16, tag="xt2", name="xt2")
            wx8_s = inp.tile([128, 2, 4 * H], fp8, tag="wx8", name="wx8")
            wx2_s = inp.tile([KDIM - 256, 4 * H], bf16, tag="wx2", name="wx2")
            wh_s = []
            whf_s = []
            for k in range(2):
                wh_s.append(inp.tile([128, 3 * H], bf16, tag=f"wh{k}",
                                     name=f"wh{k}"))
                whf_s.append(inp.tile([128, H], bf16, tag=f"whf{k}",
                                      name=f"whf{k}"))
            id_s = inp.tile([128, 128], bf16, tag="ident", name="ident")

            # ---- DMA in: ONE priority-ordered HWDGE queue (sync) so the
            # earliest-needed pieces get full HBM bandwidth, no contention.
            nc.sync.dma_start(out=wx8_s[:, :, 0:512], in_=wx8_d[:, :, 0:512])
            nc.sync.dma_start(out=xt8_s[:, :, 0:L7P], in_=xt8_d[:, :, 0:L7P])
            nc.sync.dma_start(out=xt8_s[:, :, L7P:XCOLS],
                              in_=xt8_d[:, :, L7P:XCOLS])
            nc.sync.dma_start(out=xt2_s[:], in_=xt2_d[:, :])
            nc.sync.dma_start(out=wx2_s[:], in_=wx2_d[:, :])
            nc.sync.dma_start(out=wx8_s[:, :, 512:1024],
                              in_=wx8_d[:, :, 512:1024])

            # ---- activation-table preload on the scalar queue (its table
            # DMA overlaps the input transfers' tail, before any real ACT)
            scr = wk.tile([128, 8], f32, tag="scr", name="scr")
            nc.vector.memset(scr[:], 0.0)
            nc.scalar.activation(scr[:], scr[:], Sig)
            nc.scalar.activation(scr[:], scr[:], Tanh)



            # h-side weights + identity ride the scalar queue behind the
            # table preloads (needed only when the L6i chunk starts)
            for k in range(2):
                nc.scalar.dma_start(out=whf_s[k][:],
                                    in_=whf_d[128*k:128*(k+1), :])
                nc.scalar.dma_start(out=wh_s[k][:],
                                    in_=wh_d[128*k:128*(k+1), :])
            nc.scalar.dma_start(out=id_s[:], in_=id_d[:, :])

            # ---- persistent state tiles (bf16)
            SH7 = st.tile([128, 2, L7P], bf16, tag="sh7", name="sh7")
            SC7 = st.tile([128, 2, L7P], bf16, tag="sc7", name="sc7")
            SH6 = st.tile([128, 2, NL6], bf16, tag="sh6", name="sh6")
            SC6 = st.tile([128, 2, NL6], bf16, tag="sc6", name="sc6")

            # ---- resident PSUM for internal-column gate pre-activations.
            # Value-zeroed up front; every matmul into them uses start=False
            # (accumulate-where-written / overwrite-where-not — both correct
            # on a zeroed bank regardless of scheduler order).
            res_io = pres.tile([128, 2, 2, ip], f32, tag="rio", name="rio")
            res_u = pres.tile([128, 2, ip], f32, tag="ru", name="ru")
            gf_ps = pres.tile([128, 2, ip], f32, tag="rf", name="rf")
            nc.vector.memset(res_io[:], 0.0)
            nc.vector.memset(res_u[:], 0.0)
            nc.vector.memset(gf_ps[:], 0.0)
            gf_s = st.tile([128, 2, ip], bf16, tag="gfs", name="gfs")

            def mm(out, lhsT, rhs, start, stop, pm=None):
                nc.tensor.matmul(out, lhsT, rhs, start=start, stop=stop,
                                 skip_group_check=True, perf_mode=pm)

            # --- phase-1 helpers. Packed gate order [i, u, o, f]; each
            # (gate, phi) is an fp8-DoubleRow matmul over emb rows 0:256
            # plus a 45-row bf16 tail. The two phi tails of a gate are
            # row-packed into disjoint PE row-groups (rows 0:48 / 64:112 of
            # the duplicated xt2/wx2 tiles) so they run concurrently.
            def p1_gate_outs(out0, out1, g, a, b, start):
                col = 256 * g
                mm(out0, wx8_s[:, :, col:col + 128], xt8_s[:, :, a:b],
                   start=start, stop=False, pm=DR)
                mm(out1, wx8_s[:, :, col + 128:col + 256], xt8_s[:, :, a:b],
                   start=start, stop=False, pm=DR)
                mm(out0, wx2_s[0:48, col:col + 128], xt2_s[0:48, a:b],
                   start=False, stop=True)
                mm(out1, wx2_s[0:48, col + 128:col + 256],
                   xt2_s[0:48, a:b], start=False, stop=True)

            # one gate over leaf cols [a,b) into tile P
            def p1_gate(P, g, a, b):
                n = b - a
                p1_gate_outs(P[:, 0, 0:n], P[:, 1, 0:n], g, a, b, start=True)

            # --- phase-1 into resident banks (internal cols, all 4 gates);
            # banks are pre-zeroed so every matmul uses start=False
            def p1_resident():
                a, b = RI
                p1_gate_outs(res_io[:, 0, 0, :], res_io[:, 0, 1, :], 0, a, b,
                             start=False)
                p1_gate_outs(res_io[:, 1, 0, :], res_io[:, 1, 1, :], 2, a, b,
                             start=False)
                p1_gate_outs(res_u[:, 0, :], res_u[:, 1, :], 1, a, b,
                             start=False)
                p1_gate_outs(gf_ps[:, 0, :], gf_ps[:, 1, :], 3, a, b,
                             start=False)
                # gf (x64) to SBUF bf16 for the later broadcast matmul
                nc.vector.tensor_copy(gf_s[:], gf_ps[:])

            # --- leaf ranges: PSUM gate tiles -> activations -> states
            def leaf_mm_gate(g, a, b, name):
                P = pstr.tile([128, 2, 512], f32, tag="lps", name=name)
                p1_gate(P, g, a, b)
                return P

            def leaf_act(P, n, func, tag, off):
                G = wk.tile([128, 2, n], bf16, tag=tag, name=f"{tag}{off}")
                nc.scalar.activation(G[:], P[:, :, 0:n], func, scale=INV)
                return G

            def leaf_range(a, b, SH, SC, off):
                # gate order i, u, o: c and tanh(c) complete while the
                # o-gate matmuls still stream, shortening the h tail
                n = b - a
                Pi = leaf_mm_gate(0, a, b, f"pi{a}")
                Pu = leaf_mm_gate(1, a, b, f"pu{a}")
                GI = leaf_act(Pi, n, Sig, "gi", off)
                GU = leaf_act(Pu, n, Tanh, "gu", off)
                Cd = SC[:, :, off:off + n]
                nc.vector.tensor_mul(Cd, GI[:], GU[:])
                TC = wk.tile([128, 2, n], bf16, tag="tc", name=f"tc{off}")
                nc.scalar.activation(TC[:], Cd, Tanh)
                # o-gate phi-split: h for phi0 lands while phi1 still runs
                Po = leaf_mm_gate(2, a, b, f"po{a}")
                GO = wk.tile([128, 2, n], bf16, tag="go", name=f"go{off}")
                for phi in range(2):
                    nc.scalar.activation(GO[:, phi], Po[:, phi, 0:n], Sig,
                                         scale=INV)
                    nc.vector.tensor_mul(SH[:, phi, off:off + n],
                                         GO[:, phi], TC[:, phi])

            # --- the one internal chunk: L7 (child-major) -> L6[0:96]
            # part 1: f-gate matmuls + child h-sum (needs only SH7)
            def chunk_part1():
                nf = K * ip  # 384
                Pf = pstr.tile([128, 2, 512], f32, tag="lps", name="pf")
                for phi in range(2):
                    for k in range(2):
                        mm(Pf[:, phi, 0:nf],
                           whf_s[k][:, 128 * phi:128 * phi + 128],
                           SH7[:, k, :], start=(k == 0), stop=False)
                    gbr = gf_s[:, phi, :][:, None, :].broadcast_to(
                        [128, K, ip])
                    mm(Pf[:, phi, 0:nf], id_s[:, 0:128], gbr,
                       start=False, stop=True)
                # hs = sum of 4 children (child-major: unit-stride adds,
                # split across the vector + gpsimd engines)
                cv = SH7.rearrange("p t (c j) -> p t c j", c=K)
                A = wk.tile([128, 2, ip], bf16, tag="ha", name="ha")
                nc.vector.tensor_add(A[:], cv[:, :, 0, :], cv[:, :, 1, :])
                B = wk.tile([128, 2, ip], bf16, tag="hb", name="hb")
                nc.gpsimd.tensor_add(B[:], cv[:, :, 2, :], cv[:, :, 3, :])
                hs = wk.tile([128, 2, ip], bf16, tag="hs", name="hs")
                nc.vector.tensor_add(hs[:], A[:], B[:])
                return Pf, hs

            # part 2: i/o/u h-side matmuls + activations + cell update
            def chunk_part2(Pf, hs):
                nf = K * ip
                for gi_ in range(2):
                    for phi in range(2):
                        for k in range(2):
                            mm(res_io[:, gi_, phi, :],
                               wh_s[k][:, 256 * gi_ + 128 * phi:
                                       256 * gi_ + 128 * phi + 128],
                               hs[:, k, :], start=False, stop=(k == 1))
                for phi in range(2):
                    for k in range(2):
                        mm(res_u[:, phi, :],
                           wh_s[k][:, 512 + 128 * phi:512 + 128 * phi + 128],
                           hs[:, k, :], start=False, stop=(k == 1))
                # activations
                SF = wk.tile([128, 2, nf], bf16, tag="sf", name="sf")
                nc.scalar.activation(SF[:], Pf[:, :, 0:nf], Sig, scale=INV)
                Gio = wk.tile([128, 2, 2, ip], bf16, tag="gio", name="gio")
                nc.scalar.activation(Gio[:], res_io[:], Sig, scale=INV)
                GU = wk.tile([128, 2, ip], bf16, tag="gu", name="cgu")
                nc.scalar.activation(GU[:], res_u[:], Tanh, scale=INV)
                # FS = sigmoid(FA) * c_child ; csum = sum over 4 children
                FS = wk.tile([128, 2, nf], bf16, tag="fs", name="fs")
                nc.vector.tensor_mul(FS[:], SF[:], SC7[:])
                fv = FS.rearrange("p t (c j) -> p t c j", c=K)
                CA = wk.tile([128, 2, ip], bf16, tag="ca", name="ca")
                nc.vector.tensor_add(CA[:], fv[:, :, 0, :], fv[:, :, 1, :])
                CB = wk.tile([128, 2, ip], bf16, tag="cb", name="cb")
                nc.gpsimd.tensor_add(CB[:], fv[:, :, 2, :], fv[:, :, 3, :])
                # c = ig*ug + (CA + CB) ; h = og*tanh(c)
                t1 = wk.tile([128, 2, ip], bf16, tag="t1", name="t1")
                nc.vector.tensor_mul(t1[:], Gio[:, 0], GU[:])
                t2 = wk.tile([128, 2, ip], bf16, tag="t2", name="t2")
                nc.vector.tensor_add(t2[:], t1[:], CA[:])
                Cd = SC6[:, :, 0:ip]
                nc.vector.tensor_add(Cd, t2[:], CB[:])
                nc.scalar.dma_start(out=out_c_d[:, :, 0:IPMAX],
                                    in_=SC6[:, :, 0:IPMAX])
                TC = wk.tile([128, 2, ip], bf16, tag="tc2", name="tc2")
                nc.scalar.activation(TC[:], Cd, Tanh)
                nc.vector.tensor_mul(SH6[:, :, 0:ip], Gio[:, 1], TC[:])
                nc.scalar.dma_start(out=out_h_d[:, :, 0:IPMAX],
                                    in_=SH6[:, :, 0:IPMAX])

            # ================= program order =================
            # R0 -> RI -> R1-i -> chunk f/hs -> R1-o (fills the hs wait)
            # -> chunk i/o/u + cell -> R1-u + states -> DMA out (sync queue)
            leaf_range(R0[0], R0[1], SH7, SC7, 0)
            p1_resident()
            n1 = R1[1] - R1[0]
            Pi1 = leaf_mm_gate(0, R1[0], R1[1], "pi1")
            GI1 = leaf_act(Pi1, n1, Sig, "gi", IPMAX)
            Pu1 = leaf_mm_gate(1, R1[0], R1[1], "pu1")
            Pf, hs = chunk_part1()
            GU1 = leaf_act(Pu1, n1, Tanh, "gu", IPMAX)
            Cd1 = SC6[:, :, IPMAX:NL6]
            nc.vector.tensor_mul(Cd1, GI1[:], GU1[:])
            TC1 = wk.tile([128, 2, n1], bf16, tag="tc", name="tc96")
            nc.sync.dma_start(out=out_c_d[:, :, IPMAX:NL6],
                              in_=SC6[:, :, IPMAX:NL6])
            nc.scalar.activation(TC1[:], Cd1, Tanh)
            chunk_part2(Pf, hs)
            Po1 = leaf_mm_gate(2, R1[0], R1[1], "po1")
            GO1 = wk.tile([128, 2, n1], bf16, tag="go", name="go96")
            for phi in range(2):
                nc.scalar.activation(GO1[:, phi], Po1[:, phi, 0:n1], Sig,
                                     scale=INV)
                nc.vector.tensor_mul(SH6[:, phi, IPMAX:NL6],
                                     GO1[:, phi], TC1[:, phi])
                nc.sync.dma_start(out=out_h_d[:, phi, IPMAX:NL6],
                                  in_=SH6[:, phi, IPMAX:NL6])

    nc.compile()
    return nc


def _get_compiled():
    global _COMPILED
    if _COMPILED is None:
        _COMPILED = _build_device_program()
    return _COMPILED


def _numpy_fallback(xs, child_idx, child_mask, emb_table, Wx, bx, Wh, bh,
                    Wout, bout):
    """Exact sequential scan; only used if the tree isn't the static heap."""
    X = np.asarray(emb_table, dtype=F32)[np.asarray(xs)]
    Wx = np.asarray(Wx, dtype=F32)
    Wh = np.asarray(Wh, dtype=F32)
    bx = np.asarray(bx, dtype=F32)
    bh = np.asarray(bh, dtype=F32)
    gx = np.einsum('ghd,nd->ngh', Wx, X).astype(F32) + bx
    Hb = np.zeros((N, H), dtype=F32)
    Cb = np.zeros((N, H), dtype=F32)
    ci = np.asarray(child_idx)
    cm = np.asarray(child_mask, dtype=F32)
    for i in range(N - 1, -1, -1):
        idx = ci[i]
        m = cm[i][:, None]
        Hc = Hb[idx] * m
        Cc = Cb[idx] * m
        hsum = Hc.sum(0)
        g = gx[i]
        ig = _sigmoid(g[0] + Wh[0] @ hsum + bh[0])
        og = _sigmoid(g[2] + Wh[2] @ hsum + bh[2])
        ug = np.tanh(g[3] + Wh[3] @ hsum + bh[3]).astype(F32)
        f = _sigmoid(g[1] + Hc @ Wh[1].T + bh[1])
        c = ig * ug + (f * Cc).sum(0)
        Hb[i] = og * np.tanh(c).astype(F32)
        Cb[i] = c
    logits = np.asarray(Wout, dtype=F32) @ Hb[0] + np.asarray(bout, dtype=F32)
    return _log_softmax(logits)


def kernel(xs, child_idx, child_mask, emb_table, Wx, bx, Wh, bh, Wout, bout):
    xs = np.asarray(xs)
    if not (np.array_equal(np.asarray(child_idx), _STATIC_IDX)
            and np.array_equal(np.asarray(child_mask, dtype=F32),
                               _STATIC_MASK)):
        return _numpy_fallback(xs, child_idx, child_mask, emb_table, Wx, bx,
                               Wh, bh, Wout, bout)

    from concourse.bass_utils import run_bass_kernel_spmd

    wx8, wx2, wh, whf, ident = _pack_weights(Wx, bx, Wh, bh)
    xt8s, xt2s = _pack_xt(xs, emb_table)
    in_maps = [{"xt8": xt8s[c], "xt2": xt2s[c], "wx8": wx8, "wx2": wx2,
                "wh": wh, "whf": whf, "ident": ident}
               for c in range(NCORES)]
    nc = _get_compiled()
    res = run_bass_kernel_spmd(nc, in_maps, core_ids=list(range(NCORES)))

    Hbuf = np.zeros((5461, H), dtype=F32)
    Cbuf = np.zeros((5461, H), dtype=F32)
    for c in range(NCORES):
        _, _, l6map = _PLAN[c]
        oh = np.asarray(res.results[c]["out_h"], dtype=F32)  # [128, 2, NL6]
        oc = np.asarray(res.results[c]["out_c"], dtype=F32)
        valid = l6map >= 0
        Hbuf[l6map[valid]] = np.concatenate(
            [oh[:, 0, :], oh[:, 1, :]], axis=0).T[valid]
        Cbuf[l6map[valid]] = np.concatenate(
            [oc[:, 0, :], oc[:, 1, :]], axis=0).T[valid]

    h0 = _host_top(Hbuf, Cbuf, xs, emb_table, Wx, bx, Wh, bh)
    logits = np.asarray(Wout, dtype=F32) @ h0 + np.asarray(bout, dtype=F32)
    return _log_softmax(logits)
